# revision 3
# baseline (speedup 1.0000x reference)
"""Trainium2 Bass kernel for nn_DataSelectorCGCNN.

Strategy:
  - Host: build the padded/masked per-crystal feature matrix feat [B, D]
    (the ragged gather / data-selector part -- cheap, index-bound), fold the
    b1 bias into an extra ones-row, transpose to featT [Dpad, B].
  - Device (8 NeuronCores, data-parallel over crystals): each core computes
    h = relu(featT_shard.T @ W1pad) with float32r matmuls (full-rate fp32
    streaming mode on the PE array), K accumulated in PSUM fp32.
  - Host: scores = h @ (weight_phy*wp + weight_gen*wg)  (0.02% of FLOPs),
    concat shards -> [B, 1] float32.

Device mapping per core (Option A):
  lhsT = featT tile [128 K, 128 crystals] (stationary operand)
  rhs  = W1   tile [128 K, 512 H]         (moving operand, streamed from HBM)
  out  = PSUM tile [128 crystals, 512 H], accumulated over 47 K-tiles.
"""

import numpy as np

import concourse.bacc as bacc
import concourse.mybir as mybir
import concourse.tile as tile
from concourse.bass_utils import run_bass_kernel_spmd

# Problem geometry (hardcoded per contract)
B = 4096
MAX_N = 10
FA = 92
M_NBR = 12
FN = 41
H = 2048
D = MAX_N * (FA + M_NBR * FN + M_NBR + 1)  # 5970
N_CORES = 8
BS = B // N_CORES  # 512 crystals per core
DPAD = 6016  # 47 * 128  (>= D+1; row D carries the ones/bias row)
NK = DPAD // 128  # 47
NMC = BS // 128  # 4
NN = H // 512  # 4

_MM_DT = mybir.dt.float32r
_NP_DT = np.float32

_cache = {}


def _build_nc(reps=1):
    """Build the per-core device program. reps>1 wraps the compute body in a
    hardware loop (used only for timing in test.py)."""
    nc = bacc.Bacc("TRN2", target_bir_lowering=False, debug=False,
                   num_devices=N_CORES)
    ft_d = nc.dram_tensor("featT", [DPAD, BS], _MM_DT, kind="ExternalInput")
    w1_d = nc.dram_tensor("w1", [DPAD, H], _MM_DT, kind="ExternalInput")
    h_d = nc.dram_tensor("hout", [BS, H], mybir.dt.float32,
                         kind="ExternalOutput")

    ftr = ft_d.rearrange("(k p) b -> k p b", p=128)
    w1r = w1_d.rearrange("(k p) h -> k p h", p=128)

    with tile.TileContext(nc) as tc:
        with (
            tc.tile_pool(name="ftpool", bufs=1) as ftpool,
            tc.tile_pool(name="wpool", bufs=6) as wpool,
            tc.tile_pool(name="hpool", bufs=4) as hpool,
            tc.tile_pool(name="cpool", bufs=1) as cpool,
            tc.tile_pool(name="psum", bufs=2, space="PSUM") as psumpool,
        ):
            zero_bias = cpool.tile([128, 1], mybir.dt.float32)
            nc.any.memset(zero_bias[:], 0.0)

            # featT resident in SBUF: [128, 47, 512] (96.25 KB/partition)
            ft_sb = ftpool.tile([128, NK, BS], _MM_DT)
            for k in range(NK):
                nc.sync.dma_start(ft_sb[:, k, :], ftr[k])

            def body():
                for n in range(NN):
                    psums = []
                    for mc in range(NMC):
                        pt = psumpool.tile([128, 512], mybir.dt.float32,
                                           name=f"ps{mc}", tag=f"ps{mc}")
                        psums.append(pt)
                    for k in range(NK):
                        wt = wpool.tile([128, 512], _MM_DT, name="wt", tag="wt")
                        nc.sync.dma_start(
                            wt[:], w1r[k, :, n * 512:(n + 1) * 512])
                        for mc in range(NMC):
                            nc.tensor.matmul(
                                psums[mc][:],
                                ft_sb[:, k, mc * 128:(mc + 1) * 128],
                                wt[:],
                                start=(k == 0),
                                stop=(k == NK - 1),
                            )
                    for mc in range(NMC):
                        ht = hpool.tile([128, 512], mybir.dt.float32,
                                        name="ht", tag="ht")
                        nc.scalar.activation(
                            ht[:], psums[mc][:],
                            mybir.ActivationFunctionType.Relu,
                            bias=zero_bias[:])
                        nc.sync.dma_start(
                            h_d[mc * 128:(mc + 1) * 128,
                                n * 512:(n + 1) * 512],
                            ht[:])

            if reps > 1:
                with tc.For_i(0, reps, 1):
                    body()
            else:
                body()
    nc.compile()
    return nc


def _host_features(atom_fea, nbr_fea, nbr_fea_idx, starts, lens, max_n, b1):
    """Mirror of the reference gather/pad/concat, producing featT [DPAD, B]
    with the ones/bias row at index D."""
    N = atom_fea.shape[0]
    max_n = int(max_n)
    ar = np.arange(max_n, dtype=starts.dtype)
    n_use = np.minimum(lens, max_n)
    valid = ar[None, :] < n_use[:, None]                    # [B, max_n]
    pos = np.clip(starts[:, None] + ar[None, :], 0, N - 1)  # [B, max_n]
    mask = valid.astype(np.float32)

    atom_pad = atom_fea[pos] * mask[..., None]              # [B, max_n, FA]
    nbr_pad = (nbr_fea[pos].reshape(B, max_n, M_NBR * FN)
               * mask[..., None])
    nb = nbr_fea_idx[pos] - starts[:, None, None]
    nb = np.maximum(nb, 0)
    nb = np.where(nb >= n_use[:, None, None], 0, nb)
    nb = np.where(valid[..., None], nb, 0)
    idx_feat = nb.astype(np.float32) / max_n
    node_feat = np.concatenate(
        [atom_pad, nbr_pad, idx_feat, mask[..., None]], axis=2)
    feat = node_feat.reshape(B, -1)                         # [B, D]

    featT = np.zeros((DPAD, B), dtype=np.float32)
    featT[:D, :] = feat.T
    featT[D, :] = 1.0  # bias row (pairs with b1 row appended to W1)
    return featT


def kernel(atom_fea, nbr_fea, W1, b1, wp, wg, weight_phy, weight_gen,
           nbr_fea_idx, starts, lens, max_n):
    atom_fea = np.asarray(atom_fea, dtype=np.float32)
    nbr_fea = np.asarray(nbr_fea, dtype=np.float32)
    W1 = np.asarray(W1, dtype=np.float32)
    b1 = np.asarray(b1, dtype=np.float32)
    wp = np.asarray(wp, dtype=np.float32).reshape(-1)
    wg = np.asarray(wg, dtype=np.float32).reshape(-1)
    nbr_fea_idx = np.asarray(nbr_fea_idx, dtype=np.int32)
    starts = np.asarray(starts, dtype=np.int32)
    lens = np.asarray(lens, dtype=np.int32)

    assert W1.shape == (D, H) and starts.shape[0] == B

    featT = _host_features(atom_fea, nbr_fea, nbr_fea_idx, starts, lens,
                           max_n, b1)

    w1pad = np.zeros((DPAD, H), dtype=_NP_DT)
    w1pad[:D, :] = W1
    w1pad[D, :] = b1  # bias row

    if "nc" not in _cache:
        _cache["nc"] = _build_nc(reps=1)
    nc = _cache["nc"]

    in_maps = [
        {"featT": np.ascontiguousarray(featT[:, c * BS:(c + 1) * BS]),
         "w1": w1pad}
        for c in range(N_CORES)
    ]
    res = run_bass_kernel_spmd(nc, in_maps, core_ids=list(range(N_CORES)))

    wc = (np.float32(weight_phy) * wp
          + np.float32(weight_gen) * wg).astype(np.float32)  # [H]

    scores = np.empty((B, 1), dtype=np.float32)
    for c in range(N_CORES):
        h = res.results[c]["hout"]  # [BS, H] float32
        scores[c * BS:(c + 1) * BS, 0] = h @ wc
    return scores


# revision 6
# speedup vs baseline: 1.3098x; 1.3098x over previous
"""Trainium2 Bass kernel for nn_DataSelectorCGCNN.

Strategy:
  - Host: build the padded/masked per-crystal feature matrix feat [B, D]
    (the ragged gather / data-selector part -- cheap, index-bound), fold the
    b1 bias into an extra ones-row, transpose to featT [DPAD, B], and
    pre-tile W1 into [nN, nK, 128, 512] so every device DMA is a fully
    contiguous block.
  - Device (8 NeuronCores, data-parallel over crystals): each core computes
    h = relu(featT_shard.T @ W1pad) with float32r matmuls (full-rate fp32
    streaming mode on the PE array, ~1.7e-4 absmax accuracy), K accumulated
    in PSUM fp32.
  - Host: scores = h @ (weight_phy*wp + weight_gen*wg)  (0.02% of FLOPs),
    concat shards -> [B, 1] float32.

Device mapping per core:
  lhsT = featT tile [128 K, 128 crystals] (stationary operand, SBUF-resident)
  rhs  = W1   tile [128 K, 512 H]         (moving operand, streamed from HBM)
  out  = PSUM tile [128 crystals, 512 H], accumulated over 47 K-tiles,
         evicted through ScalarE ReLU -> SBUF -> HBM.
"""

import numpy as np

import concourse.bacc as bacc
import concourse.mybir as mybir
import concourse.tile as tile
from concourse.bass_utils import run_bass_kernel_spmd

# Problem geometry (hardcoded per contract)
B = 4096
MAX_N = 10
FA = 92
M_NBR = 12
FN = 41
H = 2048
D = MAX_N * (FA + M_NBR * FN + M_NBR + 1)  # 5970
N_CORES = 8
BS = B // N_CORES  # 512 crystals per core
DPAD = 6016  # 47 * 128  (>= D+1; row D carries the ones/bias row)
NK = DPAD // 128  # 47
NMC = BS // 128  # 4
NN = H // 512  # 4

# tuning knobs (selected from on-hardware A/B)
WBUFS = 8
KFUSE = 4
HBUFS = 4

_MM_DT = mybir.dt.float32r
_NP_DT = np.float32

_cache = {}


def _build_nc(reps=1):
    """Build the per-core device program. reps>1 wraps the compute body in a
    hardware loop (used only for timing in test.py)."""
    nc = bacc.Bacc("TRN2", target_bir_lowering=False, debug=False,
                   num_devices=N_CORES)
    ft_d = nc.dram_tensor("featT", [DPAD, BS], _MM_DT, kind="ExternalInput")
    w1_d = nc.dram_tensor("w1t", [NN, NK, 128, 512], _MM_DT,
                          kind="ExternalInput")
    h_d = nc.dram_tensor("hout", [BS, H], mybir.dt.float32,
                         kind="ExternalOutput")

    ftr = ft_d.rearrange("(k p) b -> k p b", p=128)

    with tile.TileContext(nc) as tc:
        with (
            tc.tile_pool(name="ftpool", bufs=1) as ftpool,
            tc.tile_pool(name="wpool", bufs=WBUFS) as wpool,
            tc.tile_pool(name="hpool", bufs=HBUFS) as hpool,
            tc.tile_pool(name="cpool", bufs=1) as cpool,
            tc.tile_pool(name="psum", bufs=2, space="PSUM") as psumpool,
        ):
            zero_bias = cpool.tile([128, 1], mybir.dt.float32)
            nc.any.memset(zero_bias[:], 0.0)

            # featT resident in SBUF: [128, 47, 512] (96.25 KB/partition)
            ft_sb = ftpool.tile([128, NK, BS], _MM_DT)
            for k in range(NK):
                nc.sync.dma_start(ft_sb[:, k, :], ftr[k])

            def body():
                for n in range(NN):
                    psums = []
                    for mc in range(NMC):
                        pt = psumpool.tile([128, 512], mybir.dt.float32,
                                           name=f"ps{mc}", tag=f"ps{mc}")
                        psums.append(pt)
                    for k0 in range(0, NK, KFUSE):
                        klen = min(KFUSE, NK - k0)
                        wt = wpool.tile([128, KFUSE, 512], _MM_DT,
                                        name="wt", tag="wt")[:, :klen, :]
                        src = w1_d[n, k0:k0 + klen]
                        nc.sync.dma_start(wt[:], src.rearrange("a p c -> p a c"))
                        for j in range(klen):
                            k = k0 + j
                            for mc in range(NMC):
                                nc.tensor.matmul(
                                    psums[mc][:],
                                    ft_sb[:, k, mc * 128:(mc + 1) * 128],
                                    wt[:, j, :],
                                    start=(k == 0),
                                    stop=(k == NK - 1),
                                )
                    for mc in range(NMC):
                        ht = hpool.tile([128, 512], mybir.dt.float32,
                                        name="ht", tag="ht")
                        nc.scalar.activation(
                            ht[:], psums[mc][:],
                            mybir.ActivationFunctionType.Relu,
                            bias=zero_bias[:])
                        nc.sync.dma_start(
                            h_d[mc * 128:(mc + 1) * 128,
                                n * 512:(n + 1) * 512],
                            ht[:])

            if reps > 1:
                with tc.For_i(0, reps, 1):
                    body()
            else:
                body()
    nc.compile()
    return nc


def _host_features(atom_fea, nbr_fea, nbr_fea_idx, starts, lens, max_n):
    """Mirror of the reference gather/pad/concat, producing featT [DPAD, B]
    with a ones row at index D (pairs with the b1 row appended to W1)."""
    N = atom_fea.shape[0]
    max_n = int(max_n)
    ar = np.arange(max_n, dtype=starts.dtype)
    n_use = np.minimum(lens, max_n)
    valid = ar[None, :] < n_use[:, None]                    # [B, max_n]
    pos = np.clip(starts[:, None] + ar[None, :], 0, N - 1)  # [B, max_n]
    mask = valid.astype(np.float32)

    atom_pad = atom_fea[pos] * mask[..., None]              # [B, max_n, FA]
    nbr_pad = (nbr_fea[pos].reshape(B, max_n, M_NBR * FN)
               * mask[..., None])
    nb = nbr_fea_idx[pos] - starts[:, None, None]
    nb = np.maximum(nb, 0)
    nb = np.where(nb >= n_use[:, None, None], 0, nb)
    nb = np.where(valid[..., None], nb, 0)
    idx_feat = nb.astype(np.float32) / max_n
    node_feat = np.concatenate(
        [atom_pad, nbr_pad, idx_feat, mask[..., None]], axis=2)
    feat = node_feat.reshape(B, -1)                         # [B, D]

    featT = np.zeros((DPAD, B), dtype=np.float32)
    featT[:D, :] = feat.T
    featT[D, :] = 1.0  # bias row
    return featT


def _host_w1t(W1, b1):
    """Pad W1 with the b1 bias row, pre-tile to [NN, NK, 128, 512]."""
    w1pad = np.zeros((DPAD, H), dtype=np.float32)
    w1pad[:D, :] = W1
    w1pad[D, :] = b1
    return np.ascontiguousarray(
        w1pad.reshape(NK, 128, NN, 512).transpose(2, 0, 1, 3))


def kernel(atom_fea, nbr_fea, W1, b1, wp, wg, weight_phy, weight_gen,
           nbr_fea_idx, starts, lens, max_n):
    atom_fea = np.asarray(atom_fea, dtype=np.float32)
    nbr_fea = np.asarray(nbr_fea, dtype=np.float32)
    W1 = np.asarray(W1, dtype=np.float32)
    b1 = np.asarray(b1, dtype=np.float32)
    wp = np.asarray(wp, dtype=np.float32).reshape(-1)
    wg = np.asarray(wg, dtype=np.float32).reshape(-1)
    nbr_fea_idx = np.asarray(nbr_fea_idx, dtype=np.int32)
    starts = np.asarray(starts, dtype=np.int32)
    lens = np.asarray(lens, dtype=np.int32)

    assert W1.shape == (D, H) and starts.shape[0] == B

    featT = _host_features(atom_fea, nbr_fea, nbr_fea_idx, starts, lens,
                           max_n)
    w1t = _host_w1t(W1, b1)

    if "nc" not in _cache:
        _cache["nc"] = _build_nc(reps=1)
    nc = _cache["nc"]

    in_maps = [
        {"featT": np.ascontiguousarray(featT[:, c * BS:(c + 1) * BS]),
         "w1t": w1t}
        for c in range(N_CORES)
    ]
    res = run_bass_kernel_spmd(nc, in_maps, core_ids=list(range(N_CORES)))

    wc = (np.float32(weight_phy) * wp
          + np.float32(weight_gen) * wg).astype(np.float32)  # [H]

    scores = np.empty((B, 1), dtype=np.float32)
    for c in range(N_CORES):
        h = res.results[c]["hout"]  # [BS, H] float32
        scores[c * BS:(c + 1) * BS, 0] = h @ wc
    return scores


# revision 7
# speedup vs baseline: 1.3116x; 1.0014x over previous
"""Trainium2 Bass kernel for nn_DataSelectorCGCNN.

Strategy:
  - Host: build the padded/masked per-crystal feature matrix feat [B, D]
    (the ragged gather / data-selector part -- cheap, index-bound), fold the
    b1 bias into an extra ones-row, transpose to featT [DPAD, B], and
    pre-tile W1 into [nN, nK, 128, 512] so every device DMA is a fully
    contiguous block.
  - Device (8 NeuronCores, data-parallel over crystals): each core computes
    h = relu(featT_shard.T @ W1pad) with float32r matmuls (full-rate fp32
    streaming mode on the PE array, ~1.7e-4 absmax accuracy), K accumulated
    in PSUM fp32.
  - Host: scores = h @ (weight_phy*wp + weight_gen*wg)  (0.02% of FLOPs),
    concat shards -> [B, 1] float32.

Device mapping per core:
  lhsT = featT tile [128 K, 128 crystals] (stationary operand, SBUF-resident)
  rhs  = W1   tile [128 K, 512 H]         (moving operand, streamed from HBM)
  out  = PSUM tile [128 crystals, 512 H], accumulated over 47 K-tiles,
         evicted through ScalarE ReLU -> SBUF -> HBM.
"""

import os

import numpy as np

# The axon client in this container has no NTFF profile hook; make sure a
# stray BASS_TRACE in the environment can't route us onto that path.
os.environ.setdefault("BASS_NEVER_TRACE", "1")

import concourse.bacc as bacc
import concourse.mybir as mybir
import concourse.tile as tile
from concourse.bass_utils import run_bass_kernel_spmd

# Problem geometry (hardcoded per contract)
B = 4096
MAX_N = 10
FA = 92
M_NBR = 12
FN = 41
H = 2048
D = MAX_N * (FA + M_NBR * FN + M_NBR + 1)  # 5970
N_CORES = 8
BS = B // N_CORES  # 512 crystals per core
DPAD = 6016  # 47 * 128  (>= D+1; row D carries the ones/bias row)
NK = DPAD // 128  # 47
NMC = BS // 128  # 4
NN = H // 512  # 4

# tuning knobs (selected from on-hardware A/B)
WBUFS = 8
KFUSE = 4
HBUFS = 4

_MM_DT = mybir.dt.float32r
_NP_DT = np.float32

_cache = {}


def _build_nc(reps=1):
    """Build the per-core device program. reps>1 wraps the compute body in a
    hardware loop (used only for timing in test.py)."""
    nc = bacc.Bacc("TRN2", target_bir_lowering=False, debug=False,
                   num_devices=N_CORES)
    ft_d = nc.dram_tensor("featT", [DPAD, BS], _MM_DT, kind="ExternalInput")
    w1_d = nc.dram_tensor("w1t", [NN, NK, 128, 512], _MM_DT,
                          kind="ExternalInput")
    h_d = nc.dram_tensor("hout", [BS, H], mybir.dt.float32,
                         kind="ExternalOutput")

    ftr = ft_d.rearrange("(k p) b -> k p b", p=128)

    with tile.TileContext(nc) as tc:
        with (
            tc.tile_pool(name="ftpool", bufs=1) as ftpool,
            tc.tile_pool(name="wpool", bufs=WBUFS) as wpool,
            tc.tile_pool(name="hpool", bufs=HBUFS) as hpool,
            tc.tile_pool(name="cpool", bufs=1) as cpool,
            tc.tile_pool(name="psum", bufs=2, space="PSUM") as psumpool,
        ):
            zero_bias = cpool.tile([128, 1], mybir.dt.float32)
            nc.any.memset(zero_bias[:], 0.0)

            # featT resident in SBUF: [128, 47, 512] (96.25 KB/partition)
            ft_sb = ftpool.tile([128, NK, BS], _MM_DT)
            for k in range(NK):
                nc.sync.dma_start(ft_sb[:, k, :], ftr[k])

            def body():
                for n in range(NN):
                    psums = []
                    for mc in range(NMC):
                        pt = psumpool.tile([128, 512], mybir.dt.float32,
                                           name=f"ps{mc}", tag=f"ps{mc}")
                        psums.append(pt)
                    for k0 in range(0, NK, KFUSE):
                        klen = min(KFUSE, NK - k0)
                        wt = wpool.tile([128, KFUSE, 512], _MM_DT,
                                        name="wt", tag="wt")[:, :klen, :]
                        src = w1_d[n, k0:k0 + klen]
                        nc.sync.dma_start(wt[:], src.rearrange("a p c -> p a c"))
                        for j in range(klen):
                            k = k0 + j
                            for mc in range(NMC):
                                nc.tensor.matmul(
                                    psums[mc][:],
                                    ft_sb[:, k, mc * 128:(mc + 1) * 128],
                                    wt[:, j, :],
                                    start=(k == 0),
                                    stop=(k == NK - 1),
                                )
                    for mc in range(NMC):
                        ht = hpool.tile([128, 512], mybir.dt.float32,
                                        name="ht", tag="ht")
                        nc.scalar.activation(
                            ht[:], psums[mc][:],
                            mybir.ActivationFunctionType.Relu,
                            bias=zero_bias[:])
                        nc.sync.dma_start(
                            h_d[mc * 128:(mc + 1) * 128,
                                n * 512:(n + 1) * 512],
                            ht[:])

            if reps > 1:
                with tc.For_i(0, reps, 1):
                    body()
            else:
                body()
    nc.compile()
    return nc


def _host_features(atom_fea, nbr_fea, nbr_fea_idx, starts, lens, max_n):
    """Mirror of the reference gather/pad/concat, producing featT [DPAD, B]
    with a ones row at index D (pairs with the b1 row appended to W1)."""
    N = atom_fea.shape[0]
    max_n = int(max_n)
    ar = np.arange(max_n, dtype=starts.dtype)
    n_use = np.minimum(lens, max_n)
    valid = ar[None, :] < n_use[:, None]                    # [B, max_n]
    pos = np.clip(starts[:, None] + ar[None, :], 0, N - 1)  # [B, max_n]
    mask = valid.astype(np.float32)

    atom_pad = atom_fea[pos] * mask[..., None]              # [B, max_n, FA]
    nbr_pad = (nbr_fea[pos].reshape(B, max_n, M_NBR * FN)
               * mask[..., None])
    nb = nbr_fea_idx[pos] - starts[:, None, None]
    nb = np.maximum(nb, 0)
    nb = np.where(nb >= n_use[:, None, None], 0, nb)
    nb = np.where(valid[..., None], nb, 0)
    idx_feat = nb.astype(np.float32) / max_n
    node_feat = np.concatenate(
        [atom_pad, nbr_pad, idx_feat, mask[..., None]], axis=2)
    feat = node_feat.reshape(B, -1)                         # [B, D]

    featT = np.zeros((DPAD, B), dtype=np.float32)
    featT[:D, :] = feat.T
    featT[D, :] = 1.0  # bias row
    return featT


def _host_w1t(W1, b1):
    """Pad W1 with the b1 bias row, pre-tile to [NN, NK, 128, 512]."""
    w1pad = np.zeros((DPAD, H), dtype=np.float32)
    w1pad[:D, :] = W1
    w1pad[D, :] = b1
    return np.ascontiguousarray(
        w1pad.reshape(NK, 128, NN, 512).transpose(2, 0, 1, 3))


def kernel(atom_fea, nbr_fea, W1, b1, wp, wg, weight_phy, weight_gen,
           nbr_fea_idx, starts, lens, max_n):
    atom_fea = np.asarray(atom_fea, dtype=np.float32)
    nbr_fea = np.asarray(nbr_fea, dtype=np.float32)
    W1 = np.asarray(W1, dtype=np.float32)
    b1 = np.asarray(b1, dtype=np.float32)
    wp = np.asarray(wp, dtype=np.float32).reshape(-1)
    wg = np.asarray(wg, dtype=np.float32).reshape(-1)
    nbr_fea_idx = np.asarray(nbr_fea_idx, dtype=np.int32)
    starts = np.asarray(starts, dtype=np.int32)
    lens = np.asarray(lens, dtype=np.int32)

    assert W1.shape == (D, H) and starts.shape[0] == B

    featT = _host_features(atom_fea, nbr_fea, nbr_fea_idx, starts, lens,
                           max_n)
    w1t = _host_w1t(W1, b1)

    if "nc" not in _cache:
        _cache["nc"] = _build_nc(reps=1)
    nc = _cache["nc"]

    in_maps = [
        {"featT": np.ascontiguousarray(featT[:, c * BS:(c + 1) * BS]),
         "w1t": w1t}
        for c in range(N_CORES)
    ]
    res = run_bass_kernel_spmd(nc, in_maps, core_ids=list(range(N_CORES)))

    wc = (np.float32(weight_phy) * wp
          + np.float32(weight_gen) * wg).astype(np.float32)  # [H]

    scores = np.empty((B, 1), dtype=np.float32)
    for c in range(N_CORES):
        h = res.results[c]["hout"]  # [BS, H] float32
        scores[c * BS:(c + 1) * BS, 0] = h @ wc
    return scores
